# revision 1
# baseline (speedup 1.0000x reference)
"""Criss-Cross Attention TRN2 Bass kernel.

Problem: x[16,512,96,96]; q,k = 1x1 conv to 64ch; v = 1x1 conv to 512ch;
column+row criss-cross softmax attention (column set excludes the center
pixel); out = gamma * agg + x.

Sharding: data-parallel over batch, 2 batches per core on 8 cores.

Per-core dataflow (per batch):
  P: stream x (f16) in 384-pixel blocks; project q,k (weight-stationary
     f16 matmuls, fp32 psum) and v; the row-energy (eW) group for the 4
     rows each block completes is computed inline (exp + row-sum), fully
     overlapping the eW half of the softmax with the projections.
  E: column energies eH per 4-column group: PE matmul -> ACT exp (bf16,
     unnormalized) -> gpsimd affine_select zeroes the u==h center ->
     DVE row-sum.  D = SH + SW^T (PE transpose); rDg = 1/D (gamma is
     folded into Wv/bv host-side, so no gamma multiply on device).
  C: per column pair: PE-transpose v slices (psum f16) -> evac
     (DVE/ACT alternating); attn = expH * rDg (per-partition scalar);
     PE-transpose attn; agg matmuls -> psum [c,h]; evac to o_col f16.
  R: per 4-row quad (2 pairs): same transposes/agg; the o_col rows are
     accumulated into the same psum group via identity matmuls
     (start/stop chaining) instead of vector adds; combine+x-residual
     via one fused scalar_tensor_tensor (p0) or copy+add (p1); one
     f16 out-DMA per quad so every HBM run is 768B (a f16 row-pair
     would pay the <512B descriptor penalty; quads also halve the
     DMA count for the x re-read).

Output is f16; the host upcasts to f32 (rel-err budget 2e-2 vs
achieved ~1.2e-3).
"""

import numpy as np

import concourse.bass as bass
import concourse.mybir as mybir
import concourse.tile as tile
from concourse import bacc
from concourse.alu_op_type import AluOpType
from concourse.masks import make_identity

F16 = mybir.dt.float16
F32 = mybir.dt.float32
BF16 = mybir.dt.bfloat16
AF = mybir.ActivationFunctionType

B, C, H, W = 16, 512, 96, 96
CQK = 64
HW = H * W
NCORES = 8
BLOC = B // NCORES  # batches per core
KCH = 4  # C / 128 channel chunks
PXB = 384  # pixel block for projections
NPXB = HW // PXB
WG = 4  # columns/rows per energy group


def build_nc():
    nc = bacc.Bacc()

    x16 = nc.declare_dram_parameter("x16", [BLOC, C, H, W], F16, isOutput=False)
    wqkT = nc.declare_dram_parameter("wqkT", [C, 2 * CQK], F16, isOutput=False)
    wvT = nc.declare_dram_parameter("wvT", [C, C], F16, isOutput=False)
    bqk = nc.declare_dram_parameter("bqk", [2 * CQK], F32, isOutput=False)
    bv = nc.declare_dram_parameter("bv", [C], F32, isOutput=False)
    gamma = nc.declare_dram_parameter("gamma", [1], F32, isOutput=False)
    out = nc.declare_dram_parameter("out", [BLOC, C, H, W], F16, isOutput=True)

    x16ap = x16[:]
    outap = out[:]

    with tile.TileContext(nc) as tc:
        with (
            tc.tile_pool(name="cn", bufs=1) as cn,
            tc.tile_pool(name="big", bufs=1) as big,
            tc.tile_pool(name="att", bufs=1) as att,
            tc.tile_pool(name="sm", bufs=1) as sm,
            tc.tile_pool(name="st", bufs=2) as st,
            tc.tile_pool(name="wkv", bufs=2) as wkv,
            tc.tile_pool(name="wk", bufs=2) as wk,
            tc.tile_pool(name="pp", bufs=2, space="PSUM") as pp,
        ):
            # ---- constants ----
            id128 = cn.tile([128, 128], F16, tag="id128")
            make_identity(nc, id128)
            id96f = cn.tile([96, 96], F32, tag="id96f")
            make_identity(nc, id96f)

            wqkT_sb = cn.tile([128, KCH, 2 * CQK], F16, tag="wqkT")
            nc.sync.dma_start(
                out=wqkT_sb,
                in_=bass.AP(
                    tensor=wqkT[:].tensor,
                    offset=wqkT[:].offset,
                    ap=[[2 * CQK, 128], [128 * 2 * CQK, KCH], [1, 2 * CQK]],
                ),
            )
            wvT_sb = cn.tile([128, KCH, C], F16, tag="wvT")
            nc.sync.dma_start(
                out=wvT_sb,
                in_=bass.AP(
                    tensor=wvT[:].tensor,
                    offset=wvT[:].offset,
                    ap=[[C, 128], [128 * C, KCH], [1, C]],
                ),
            )
            bq_sb = cn.tile([CQK, 1], F32, tag="bq")
            nc.sync.dma_start(
                out=bq_sb,
                in_=bass.AP(
                    tensor=bqk[:].tensor, offset=bqk[:].offset, ap=[[1, CQK], [1, 1]]
                ),
            )
            bk_sb = cn.tile([CQK, 1], F32, tag="bk")
            nc.sync.dma_start(
                out=bk_sb,
                in_=bass.AP(
                    tensor=bqk[:].tensor,
                    offset=bqk[:].offset + CQK,
                    ap=[[1, CQK], [1, 1]],
                ),
            )
            bv_sb = cn.tile([128, KCH], F32, tag="bv")
            nc.sync.dma_start(
                out=bv_sb,
                in_=bass.AP(
                    tensor=bv[:].tensor, offset=bv[:].offset, ap=[[1, 128], [128, KCH]]
                ),
            )
            gam_sb = cn.tile([96, 1], F32, tag="gam")
            nc.sync.dma_start(
                out=gam_sb,
                in_=bass.AP(
                    tensor=gamma[:].tensor, offset=gamma[:].offset, ap=[[0, 96], [1, 1]]
                ),
            )

            for b in range(BLOC):
                # ---------- Phase P: projections (+ interleaved eW groups) ----------
                qk_sb = big.tile([CQK, 2, HW], F16, tag="big")  # [:,0]=q, [:,1]=k
                v_sb = big.tile([128, KCH, H, W], F16, tag="v")
                expH = att.tile([96, W, H], BF16, tag="eh")  # [h, w, u]
                expW = att.tile([96, H, W], BF16, tag="ew")  # [w, h, v]
                SH = sm.tile([96, W], F32, tag="SH")  # [h, w]
                SW = sm.tile([96, H], F32, tag="SW")  # [w, h]
                rDg = sm.tile([96, W], F32, tag="rDg")  # 1/D (gamma folded in Wv)
                q3 = qk_sb[:, 0, :].rearrange("c (h w) -> c h w", w=W)
                k3 = qk_sb[:, 1, :].rearrange("c (h w) -> c h w", w=W)
                for j in range(NPXB):
                    xs = st.tile([128, KCH, PXB], F16, tag="xs")
                    nc.sync.dma_start(
                        out=xs,
                        in_=bass.AP(
                            tensor=x16ap.tensor,
                            offset=x16ap.offset + b * C * HW + j * PXB,
                            ap=[[HW, 128], [128 * HW, KCH], [1, PXB]],
                        ),
                    )
                    pq = pp.tile([CQK, PXB], F32, tag="pT")
                    for k in range(KCH):
                        nc.tensor.matmul(
                            pq,
                            wqkT_sb[:, k, 0:CQK],
                            xs[:, k, :],
                            start=(k == 0),
                            stop=(k == KCH - 1),
                        )
                    nc.scalar.activation(
                        out=qk_sb[:, 0, j * PXB : (j + 1) * PXB],
                        in_=pq,
                        func=AF.Identity,
                        bias=bq_sb,
                    )
                    pk = pp.tile([CQK, PXB], F32, tag="pT")
                    for k in range(KCH):
                        nc.tensor.matmul(
                            pk,
                            wqkT_sb[:, k, CQK : 2 * CQK],
                            xs[:, k, :],
                            start=(k == 0),
                            stop=(k == KCH - 1),
                        )
                    nc.vector.tensor_scalar_add(
                        qk_sb[:, 1, j * PXB : (j + 1) * PXB], pk, bk_sb
                    )
                    for m in range(KCH):
                        pv = pp.tile([128, PXB], F32, tag="pCE")
                        for k in range(KCH):
                            nc.tensor.matmul(
                                pv,
                                wvT_sb[:, k, 128 * m : 128 * (m + 1)],
                                xs[:, k, :],
                                start=(k == 0),
                                stop=(k == KCH - 1),
                            )
                        vdst = v_sb.rearrange("p m h w -> p m (h w)")[
                            :, m, j * PXB : (j + 1) * PXB
                        ]
                        if m % 2 == 0:
                            nc.vector.tensor_scalar_add(vdst, pv, bv_sb[:, m : m + 1])
                        else:
                            nc.scalar.activation(
                                out=vdst,
                                in_=pv,
                                func=AF.Identity,
                                bias=bv_sb[:, m : m + 1],
                            )
                    # eW energy group for the 4 rows this block just produced
                    pe = pp.tile([96, WG, 96], F32, tag="pCE")
                    for i in range(WG):
                        h = j * WG + i
                        nc.tensor.matmul(
                            pe[:, i, :], q3[:, h, :], k3[:, h, :], start=True, stop=True
                        )
                    dst = expW[:, j * WG : (j + 1) * WG, :]
                    nc.scalar.activation(out=dst, in_=pe, func=AF.Exp)
                    nc.vector.tensor_reduce(
                        out=SW[:, j * WG : (j + 1) * WG],
                        in_=dst,
                        op=AluOpType.add,
                        axis=mybir.AxisListType.X,
                    )

                # ---------- Phase E: column energies (incremental rDg) ----------
                for eg in range(W // WG):
                    pe = pp.tile([96, WG, 96], F32, tag="pCE")
                    for i in range(WG):
                        w = eg * WG + i
                        nc.tensor.matmul(
                            pe[:, i, :], q3[:, :, w], k3[:, :, w], start=True, stop=True
                        )
                    dst = expH[:, eg * WG : (eg + 1) * WG, :]
                    nc.scalar.activation(out=dst, in_=pe, func=AF.Exp)
                    nc.gpsimd.affine_select(
                        out=dst,
                        in_=dst,
                        compare_op=AluOpType.not_equal,
                        fill=0.0,
                        base=0,
                        pattern=[[0, WG], [-1, 96]],
                        channel_multiplier=1,
                    )
                    nc.vector.tensor_reduce(
                        out=SH[:, eg * WG : (eg + 1) * WG],
                        in_=dst,
                        op=AluOpType.add,
                        axis=mybir.AxisListType.X,
                    )
                # D = SH + SW^T ; rDg = 1/D (gamma folded into Wv host-side)
                pt = pp.tile([96, 96], F32, tag="pCE")
                nc.tensor.transpose(pt, SW, id96f)
                nc.vector.tensor_tensor(out=SH, in0=SH, in1=pt, op=AluOpType.add)
                nc.vector.reciprocal(rDg, SH)
                pt2 = pp.tile([96, 96], F32, tag="pCE")
                nc.tensor.transpose(pt2, rDg, id96f)
                rDgT = sm.tile([96, H], F32, tag="rDgT")
                nc.vector.tensor_copy(rDgT, pt2)

                # ---------- Phase C: column pass (pairs of columns) ----------
                o_col = big.tile([128, KCH, H, W], F16, tag="big")
                for g in range(W // 2):
                    w0 = 2 * g
                    pvt = pp.tile([96, 2, KCH, 128], F16, tag="pT")
                    for wi in range(2):
                        for k in range(KCH):
                            nc.tensor.transpose(
                                pvt[:, wi, k, :], v_sb[:, k, :, w0 + wi], id128
                            )
                    vt1 = wkv.tile([96, 2, KCH, 128], F16, tag="vt")
                    if g % 2 == 0:
                        nc.vector.tensor_copy(vt1, pvt)
                    else:
                        nc.scalar.copy(vt1, pvt)
                    attn = wk.tile([96, 2, 96], F16, tag="attn")
                    for wi in range(2):
                        nc.vector.tensor_scalar_mul(
                            attn[:, wi, :],
                            expH[:, w0 + wi, :],
                            rDg[:, w0 + wi : w0 + wi + 1],
                        )
                    pat = pp.tile([96, 2, 96], F16, tag="pCE")
                    for wi in range(2):
                        nc.tensor.transpose(
                            pat[:, wi, :], attn[:, wi, :], id128[0:96, 0:96]
                        )
                    attT = wk.tile([96, 2, 96], F16, tag="attT")
                    if g % 2 == 0:
                        nc.scalar.copy(attT, pat)
                    else:
                        nc.vector.tensor_copy(attT, pat)
                    pagg = pp.tile([128, KCH, 2, 128], F32, tag="pAGG")
                    for m in range(KCH):
                        for wi in range(2):
                            nc.tensor.matmul(
                                pagg[:, m, wi, 0:96],
                                vt1[:, wi, m, :],
                                attT[:, wi, :],
                                start=True,
                                stop=True,
                            )
                    srcA = pagg[:, :, :, 0:96].rearrange("p m wi h -> p m h wi")
                    dstA = o_col[:, :, :, w0 : w0 + 2]
                    if g % 2 == 0:
                        nc.scalar.copy(dstA, srcA)
                    else:
                        nc.vector.tensor_copy(dstA, srcA)

                # ---------- Phase R: row pass (quads = 2 pairs of rows) ----------
                for q in range(H // 4):
                    h0q = 4 * q
                    orow = st.tile([128, KCH, 4, 96], F16, tag="orow")
                    xrow = st.tile([128, KCH, 384], F16, tag="xs")
                    nc.sync.dma_start(
                        out=xrow,
                        in_=bass.AP(
                            tensor=x16ap.tensor,
                            offset=x16ap.offset + b * C * HW + h0q * W,
                            ap=[[HW, 128], [128 * HW, KCH], [1, 384]],
                        ),
                    )
                    for p in range(2):
                        h0 = h0q + 2 * p
                        pvt = pp.tile([96, 2, KCH, 128], F16, tag="pT")
                        for hi in range(2):
                            for k in range(KCH):
                                nc.tensor.transpose(
                                    pvt[:, hi, k, :], v_sb[:, k, h0 + hi, :], id128
                                )
                        vt2 = wkv.tile([96, 2, KCH, 128], F16, tag="vt")
                        if p == 0:
                            nc.vector.tensor_copy(vt2, pvt)
                        else:
                            nc.scalar.copy(vt2, pvt)
                        attn2 = wk.tile([96, 2, 96], F16, tag="attn")
                        for hi in range(2):
                            nc.vector.tensor_scalar_mul(
                                attn2[:, hi, :],
                                expW[:, h0 + hi, :],
                                rDgT[:, h0 + hi : h0 + hi + 1],
                            )
                        pat2 = pp.tile([96, 2, 96], F16, tag="pCE")
                        for hi in range(2):
                            nc.tensor.transpose(
                                pat2[:, hi, :], attn2[:, hi, :], id128[0:96, 0:96]
                            )
                        attT2 = wk.tile([96, 2, 96], F16, tag="attT")
                        if p == 0:
                            nc.scalar.copy(attT2, pat2)
                        else:
                            nc.vector.tensor_copy(attT2, pat2)
                        pagg2 = pp.tile([128, KCH, 2, 128], F32, tag="pAGG")
                        for m in range(KCH):
                            for hi in range(2):
                                nc.tensor.matmul(
                                    pagg2[:, m, hi, 0:96],
                                    vt2[:, hi, m, :],
                                    attT2[:, hi, :],
                                    start=True,
                                    stop=False,
                                )
                                # accumulate the column-pass rows in PSUM
                                nc.tensor.matmul(
                                    pagg2[:, m, hi, 0:96],
                                    id128,
                                    o_col[:, m, h0 + hi, :],
                                    start=False,
                                    stop=True,
                                )
                        odst = orow[:, :, 2 * p : 2 * p + 2, :]
                        xsl = xrow.rearrange("p m (hi w) -> p m hi w", hi=4)[
                            :, :, 2 * p : 2 * p + 2, :
                        ]
                        if p == 0:
                            # fused: orow = (pagg2 * 1) + xrow in one DVE pass
                            nc.vector.scalar_tensor_tensor(
                                out=odst,
                                in0=pagg2[:, :, :, 0:96],
                                scalar=1.0,
                                in1=xsl,
                                op0=AluOpType.mult,
                                op1=AluOpType.add,
                            )
                        else:
                            nc.scalar.copy(odst, pagg2[:, :, :, 0:96])
                            nc.vector.tensor_tensor(
                                out=odst, in0=odst, in1=xsl, op=AluOpType.add
                            )
                    nc.sync.dma_start(
                        out=bass.AP(
                            tensor=outap.tensor,
                            offset=outap.offset + b * C * HW + h0q * W,
                            ap=[[HW, 128], [128 * HW, KCH], [1, 384]],
                        ),
                        in_=orow.rearrange("p m hi w -> p m (hi w)"),
                    )
    nc.finalize()
    return nc


_NC_CACHE = {}


def _get_nc():
    if "nc" not in _NC_CACHE:
        _NC_CACHE["nc"] = build_nc()
    return _NC_CACHE["nc"]


def make_in_maps(x, Wq, bq, Wk, bk, Wv, bv, gamma):
    x = np.asarray(x, dtype=np.float32)
    gamma = np.asarray(gamma, dtype=np.float32)
    g0 = float(gamma[0])
    wqkT = np.ascontiguousarray(
        np.concatenate([np.asarray(Wq), np.asarray(Wk)], axis=0).T
    ).astype(np.float16)
    # gamma folded into the value projection: out = (att @ (g*v)) + x
    wvT = np.ascontiguousarray(np.asarray(Wv).T * g0).astype(np.float16)
    bqk = np.concatenate([np.asarray(bq), np.asarray(bk)]).astype(np.float32)
    bv = np.asarray(bv, dtype=np.float32) * g0
    x16 = x.astype(np.float16)
    in_maps = []
    for c in range(NCORES):
        sl = slice(c * BLOC, (c + 1) * BLOC)
        in_maps.append(
            {
                "x16": x16[sl],
                "wqkT": wqkT,
                "wvT": wvT,
                "bqk": bqk,
                "bv": bv,
                "gamma": gamma,
            }
        )
    return in_maps


def kernel(x, Wq, bq, Wk, bk, Wv, bv, gamma):
    from concourse.bass_utils import run_bass_kernel_spmd

    nc = _get_nc()
    in_maps = make_in_maps(x, Wq, bq, Wk, bk, Wv, bv, gamma)
    res = run_bass_kernel_spmd(nc, in_maps, core_ids=list(range(NCORES)))
    return np.concatenate([r["out"] for r in res.results], axis=0).astype(np.float32)

